# revision 1
# baseline (speedup 1.0000x reference)
"""Trainium2 Bass kernel for a ViT-style transformer block (sparse_attention).

Strategy: data-parallel over batch B=32 across 8 cores (4 items/core), no
collectives. All matmuls bf16 with f32 PSUM accumulation. Attention computed
as S^T = K·Q^T (feature-major q/k), softmax denominators via an appended
ones-column in V (PV matmul yields both attention output and row sums),
normalization via PE ones-matmul broadcast of the denominator + fast
reciprocal at partition base 0.

v2: software pipeline — the proj/LN2/fc1/fc2 work of item i-1 is emitted in
small units interleaved into the attention head loop of item i, so the
TensorEngine never idles (and never drops to its cold 1.2 GHz clock) while
DVE/ACT run the softmax elementwise chain.

Host-side folding: LN gammas into the following weight matrices, LN betas into
qkv/fc1 biases, v-bias into the proj bias, attention scale into w_q.
"""

import sys

sys.path.insert(0, "/opt/trn_rl_repo")

import numpy as np
import ml_dtypes

import concourse.bass as bass
import concourse.tile as tile
from concourse import bacc, mybir
from concourse import bass_utils
from concourse.masks import make_identity



F32 = mybir.dt.float32
BF16 = mybir.dt.bfloat16

B = 32
N = 577
D = 768
H = 12
DH = 64
HID = 3072
DCH = D // 128          # 6 chunks of the model dim
HCH = HID // 128        # 24 chunks of the hidden dim
NCORES = 8
IPC = B // NCORES       # items per core
TOK = IPC * N           # tokens per core

NT = [(0, 128), (128, 128), (256, 128), (384, 128), (512, 65)]
EPS = 1e-5


def _nsplits(total):
    out = []
    o = 0
    while o < total:
        w = min(512, total - o)
        out.append((o, w))
        o += w
    return out


SPL_N = _nsplits(N)
SPL_D = _nsplits(D)
AF = mybir.ActivationFunctionType
ALU = mybir.AluOpType


def build_nc(use_bias_mm=True):
    nc = bacc.Bacc("TRN2", target_bir_lowering=False, debug=False, num_devices=NCORES)

    x_d = nc.dram_tensor("x", [TOK, D], F32, kind="ExternalInput").ap()
    maskt_d = nc.dram_tensor("maskt", [N, N], BF16, kind="ExternalInput").ap()
    wq_d = nc.dram_tensor("wq", [D, D], BF16, kind="ExternalInput").ap()
    wk_d = nc.dram_tensor("wk", [D, D], BF16, kind="ExternalInput").ap()
    wv_d = nc.dram_tensor("wv", [D, D], BF16, kind="ExternalInput").ap()
    bq_d = nc.dram_tensor("bq", [D], F32, kind="ExternalInput").ap()
    bk_d = nc.dram_tensor("bk", [D], F32, kind="ExternalInput").ap()
    wproj_d = nc.dram_tensor("wproj", [D, D], BF16, kind="ExternalInput").ap()
    wfc1_d = nc.dram_tensor("wfc1", [D, HID], BF16, kind="ExternalInput").ap()
    bfc1_d = nc.dram_tensor("bfc1", [HID], F32, kind="ExternalInput").ap()
    wfc2_d = nc.dram_tensor("wfc2", [HID, D], BF16, kind="ExternalInput").ap()
    if use_bias_mm:
        bprojr_d = nc.dram_tensor("bprojr", [D], BF16, kind="ExternalInput").ap()
        bfc2r_d = nc.dram_tensor("bfc2r", [D], BF16, kind="ExternalInput").ap()
    out_d = nc.dram_tensor("out", [TOK, D], F32, kind="ExternalOutput").ap()
    r1_d = nc.dram_tensor("r1scratch", [TOK, D], F32).ap()

    with tile.TileContext(nc) as tc:
        with (
            tc.tile_pool(name="const", bufs=1) as const,
            tc.tile_pool(name="work", bufs=1) as work,
            tc.tile_pool(name="psum", bufs=1, space="PSUM") as psum,
        ):
            # ---- constants / weights (resident) ----
            wq_sb = const.tile([128, DCH, D], BF16, name="wq_sb")
            nc.sync.dma_start(out=wq_sb, in_=wq_d.rearrange("(c p) m -> p c m", p=128))
            wk_sb = const.tile([128, DCH, D], BF16, name="wk_sb")
            nc.sync.dma_start(out=wk_sb, in_=wk_d.rearrange("(c p) m -> p c m", p=128))
            wv_sb = const.tile([128, DCH, D], BF16, name="wv_sb")
            nc.sync.dma_start(out=wv_sb, in_=wv_d.rearrange("(c p) m -> p c m", p=128))
            wproj_sb = const.tile([128, DCH, D], BF16, name="wproj_sb")
            wfc2_sb = const.tile([128, HCH, D], BF16, name="wfc2_sb")
            wfc1_r = wfc1_d.rearrange("(c p) m -> p c m", p=128)

            bq_sb = const.tile([128, DCH], F32, name="bq_sb")
            nc.sync.dma_start(out=bq_sb, in_=bq_d.rearrange("(c p) -> p c", p=128))
            bk_sb = const.tile([128, DCH], F32, name="bk_sb")
            nc.sync.dma_start(out=bk_sb, in_=bk_d.rearrange("(c p) -> p c", p=128))
            bfc1_sb = const.tile([128, HCH], F32, name="bfc1_sb")
            nc.sync.dma_start(out=bfc1_sb, in_=bfc1_d.rearrange("(c p) -> p c", p=128))
            if use_bias_mm:
                bprojr_sb = const.tile([1, D], BF16, name="bprojr_sb")
                nc.sync.dma_start(out=bprojr_sb, in_=bprojr_d[None, :])
                bfc2r_sb = const.tile([1, D], BF16, name="bfc2r_sb")
                nc.sync.dma_start(out=bfc2r_sb, in_=bfc2r_d[None, :])
                ones_row = const.tile([1, N], BF16, name="ones_row")
                nc.vector.memset(ones_row, 1.0)

            maskt_sb = const.tile([128, 5, N], BF16, name="maskt_sb")
            nc.gpsimd.memset(maskt_sb[:, 4, :], 0.0)
            for mt, (mo, msz) in enumerate(NT):
                nc.sync.dma_start(out=maskt_sb[:msz, mt, :], in_=maskt_d[mo:mo + msz, :])

            ident = const.tile([128, 128], BF16, name="ident")
            make_identity(nc, ident)
            ones128 = const.tile([128, 128], BF16, name="ones128")
            nc.gpsimd.memset(ones128, 1.0)
            eps_sb = const.tile([128, 1], F32, name="eps_sb")
            nc.vector.memset(eps_sb, EPS)

            def layernorm_tp(src, dst_fm, it, t, tsz, o, ph, cp_eng=None):
                """LN(src[tsz, D]) -> bf16 -> PE transpose -> dst_fm[:, :, o:o+tsz]."""
                stats = work.tile([128, 3, 6], F32, name=f"st{ph}_{it}_{t}", tag="stats", bufs=3)
                nc.vector.bn_stats(out=stats[:tsz, 0, :], in_=src[:, 0:256])
                nc.vector.bn_stats(out=stats[:tsz, 1, :], in_=src[:, 256:512])
                nc.vector.bn_stats(out=stats[:tsz, 2, :], in_=src[:, 512:768])
                mv = work.tile([128, 2], F32, name=f"mv{ph}_{it}_{t}", tag="mv", bufs=3)
                nc.vector.bn_aggr(out=mv[:tsz], in_=stats[:tsz])
                rstd = work.tile([128, 1], F32, name=f"rs{ph}_{it}_{t}", tag="rstd", bufs=3)
                nc.scalar.activation(out=rstd[:tsz], in_=mv[:tsz, 1:2],
                                     func=AF.Sqrt, bias=eps_sb[:tsz], scale=1.0)
                nc.vector.reciprocal(out=rstd[:tsz], in_=rstd[:tsz])
                ht_tm = work.tile([128, D], BF16, name=f"htm{ph}_{it}_{t}", tag="htm", bufs=2)
                nc.vector.tensor_scalar(out=ht_tm[:tsz], in0=src,
                                        scalar1=mv[:tsz, 0:1], scalar2=rstd[:tsz],
                                        op0=ALU.subtract, op1=ALU.mult)
                tp_ps = psum.tile([128, D], BF16, name=f"tp{ph}_{it}_{t}", tag="small", bufs=2)
                for c in range(DCH):
                    nc.tensor.transpose(tp_ps[:, c * 128:c * 128 + tsz],
                                        ht_tm[:tsz, c * 128:(c + 1) * 128],
                                        ident[:tsz, :tsz])
                if cp_eng == "dve":
                    nc.vector.tensor_copy(out=dst_fm[:, :, o:o + tsz],
                                          in_=tp_ps.rearrange("p (c q) -> p c q", c=DCH)[:, :, :tsz])
                else:
                    nc.scalar.activation(out=dst_fm[:, :, o:o + tsz],
                                         in_=tp_ps.rearrange("p (c q) -> p c q", c=DCH)[:, :, :tsz],
                                         func=AF.Copy)

            def emit_A(it):
                """x load, LN1, hT, q/k/v, vplus for item `it`."""
                t0 = it * N
                hT = work.tile([128, DCH, N], BF16, name=f"hT_{it}", tag="fmbuf", bufs=2)
                for t, (o, tsz) in enumerate(NT):
                    xt = work.tile([128, D], F32, name=f"xin_{it}_{t}", tag="xin", bufs=3)
                    nc.sync.dma_start(out=xt[:tsz, :], in_=x_d[t0 + o:t0 + o + tsz, :])
                    layernorm_tp(xt[:tsz, :], hT, it, t, tsz, o, 1)

                # q_z: per-head q with the other head-half zeroed (K=128 S^T)
                q_z = work.tile([128, H, N], BF16, name=f"q_{it}", tag="qbuf")
                nc.gpsimd.memset(q_z, 0.0)
                # k padded to 640 cols; cols 577:640 zero (tail-tile K padding)
                k_sb = work.tile([128, DCH, 640], BF16, name=f"k_{it}", tag="kbuf")
                nc.gpsimd.memset(k_sb, 0.0)
                for mc in range(DCH):
                    ps = psum.tile([128, N], F32, name=f"psq_{it}_{mc}", tag="big", bufs=3)
                    for kc in range(DCH):
                        for (o, w) in SPL_N:
                            nc.tensor.matmul(ps[:, o:o + w],
                                             wq_sb[:, kc, mc * 128:(mc + 1) * 128],
                                             hT[:, kc, o:o + w],
                                             start=(kc == 0), stop=(kc == DCH - 1))
                    nc.vector.tensor_scalar(out=q_z[0:64, 2 * mc, :], in0=ps[0:64],
                                            scalar1=bq_sb[0:64, mc:mc + 1], scalar2=None,
                                            op0=ALU.add)
                    nc.vector.tensor_scalar(out=q_z[64:128, 2 * mc + 1, :], in0=ps[64:128],
                                            scalar1=bq_sb[64:128, mc:mc + 1], scalar2=None,
                                            op0=ALU.add)
                for mc in range(DCH):
                    ps = psum.tile([128, N], F32, name=f"psk_{it}_{mc}", tag="big", bufs=3)
                    for kc in range(DCH):
                        for (o, w) in SPL_N:
                            nc.tensor.matmul(ps[:, o:o + w],
                                             wk_sb[:, kc, mc * 128:(mc + 1) * 128],
                                             hT[:, kc, o:o + w],
                                             start=(kc == 0), stop=(kc == DCH - 1))
                    nc.vector.tensor_scalar(out=k_sb[:, mc, 0:N], in0=ps,
                                            scalar1=bk_sb[:, mc:mc + 1], scalar2=None,
                                            op0=ALU.add)

                vplus = []
                for t, (o, tsz) in enumerate(NT):
                    vpe = work.tile([128, DCH, 65], BF16, name=f"vpe_{it}_{t}", tag=f"vpe{t}")
                    vpo = work.tile([128, DCH, 128], BF16, name=f"vpo_{it}_{t}", tag=f"vpo{t}")
                    for vp in (vpe, vpo):
                        if tsz < 128:
                            nc.gpsimd.memset(vp[64:128], 0.0)
                            nc.gpsimd.memset(vp[0:tsz], 1.0)
                        else:
                            nc.gpsimd.memset(vp, 1.0)
                    ps = psum.tile([128, D], F32, name=f"psv_{it}_{t}", tag="big", bufs=3)
                    for kc in range(DCH):
                        for (o2, w2) in SPL_D:
                            nc.tensor.matmul(ps[:tsz, o2:o2 + w2],
                                             hT[:, kc, o:o + tsz],
                                             wv_sb[:, kc, o2:o2 + w2],
                                             start=(kc == 0), stop=(kc == DCH - 1))
                    ev_out = bass.AP(tensor=vpe.tensor, offset=vpe.offset,
                                     ap=[vpe.ap[0], [65, 6], [1, 64]])
                    od_out = bass.AP(tensor=vpo.tensor, offset=vpo.offset + 64,
                                     ap=[vpo.ap[0], [128, 6], [1, 64]])
                    ev_in = bass.AP(tensor=ps.tensor, offset=ps.offset,
                                    ap=[ps.ap[0], [128, 6], [1, 64]])
                    od_in = bass.AP(tensor=ps.tensor, offset=ps.offset + 64,
                                    ap=[ps.ap[0], [128, 6], [1, 64]])
                    nc.scalar.activation(out=ev_out[:tsz], in_=ev_in[:tsz], func=AF.Copy)
                    nc.scalar.activation(out=od_out[:tsz], in_=od_in[:tsz], func=AF.Copy)
                    vplus.append((vpe, vpo))
                return q_z, k_sb, vplus

            def make_C_units(it, attn):
                """proj subunits, ln2 list, fc1 list, fc2 subunits for item."""
                st = {}

                def proj_u(t, o, tsz, o2, w2, last):
                    def f():
                        if f"r1_{t}" not in st:
                            st[f"r1_{t}"] = work.tile([128, D], F32, name=f"r1_{it}_{t}",
                                                      tag="r1t", bufs=3)
                            xr = work.tile([128, D], F32, name=f"xr_{it}_{t}", tag="xin", bufs=3)
                            t0 = it * N
                            nc.sync.dma_start(out=xr[:tsz, :], in_=x_d[t0 + o:t0 + o + tsz, :])
                            st[f"xr_{t}"] = xr
                        ps = psum.tile([128, w2], F32, name=f"pspj_{it}_{t}_{o2}",
                                       tag="small", bufs=2)
                        for kc in range(DCH):
                            nc.tensor.matmul(ps[:tsz, :],
                                             attn[:, kc, o:o + tsz],
                                             wproj_sb[:, kc, o2:o2 + w2],
                                             start=(kc == 0),
                                             stop=(kc == DCH - 1 and not use_bias_mm))
                        if use_bias_mm:
                            nc.tensor.matmul(ps[:tsz, :], ones_row[0:1, o:o + tsz],
                                             bprojr_sb[0:1, o2:o2 + w2],
                                             start=False, stop=True)
                        nc.vector.tensor_tensor(out=st[f"r1_{t}"][:tsz, o2:o2 + w2],
                                                in0=ps[:tsz], in1=st[f"xr_{t}"][:tsz, o2:o2 + w2],
                                                op=ALU.add)
                        if last:
                            t0 = it * N
                            nc.sync.dma_start(out=r1_d[t0 + o:t0 + o + tsz, :],
                                              in_=st[f"r1_{t}"][:tsz, :])
                    return f

                def ln2_t(t, o, tsz):
                    def f():
                        if "h2T" not in st:
                            st["h2T"] = work.tile([128, DCH, N], BF16, name=f"h2T_{it}",
                                                  tag="fmbuf", bufs=2)
                        layernorm_tp(st[f"r1_{t}"][:tsz, :], st["h2T"], it, t, tsz, o, 2)
                    return f

                def fc1_mc(mc):
                    def f():
                        if "gelu" not in st:
                            st["gelu"] = work.tile([128, HCH, N], BF16, name=f"g_{it}", tag="gelu")
                        w1c = work.tile([128, DCH, 128], BF16, name=f"w1c_{it}_{mc}",
                                        tag="w1c", bufs=2)
                        nc.sync.dma_start(out=w1c, in_=wfc1_r[:, :, mc * 128:(mc + 1) * 128])
                        ps = psum.tile([128, N], F32, name=f"psf1_{it}_{mc}", tag="big", bufs=3)
                        for kc in range(DCH):
                            for (o, w) in SPL_N:
                                nc.tensor.matmul(ps[:, o:o + w],
                                                 w1c[:, kc, :],
                                                 st["h2T"][:, kc, o:o + w],
                                                 start=(kc == 0), stop=(kc == DCH - 1))
                        nc.scalar.activation(out=st["gelu"][:, mc, :], in_=ps,
                                             func=AF.Gelu,
                                             bias=bfc1_sb[:, mc:mc + 1], scale=1.0)
                    return f

                def fc2_u(t, o, tsz, o2, w2, first):
                    def f():
                        t0 = it * N
                        if first:
                            r1r = work.tile([128, D], F32, name=f"r1r_{it}_{t}", tag="r1r", bufs=2)
                            nc.sync.dma_start(out=r1r[:tsz, :], in_=r1_d[t0 + o:t0 + o + tsz, :])
                            st[f"r1r_{t}"] = r1r
                        r1r = st[f"r1r_{t}"]
                        ps = psum.tile([128, w2], F32, name=f"psf2_{it}_{t}_{o2}",
                                       tag="small", bufs=2)
                        for kc in range(HCH):
                            nc.tensor.matmul(ps[:tsz, :],
                                             st["gelu"][:, kc, o:o + tsz],
                                             wfc2_sb[:, kc, o2:o2 + w2],
                                             start=(kc == 0),
                                             stop=(kc == HCH - 1 and not use_bias_mm))
                        if use_bias_mm:
                            nc.tensor.matmul(ps[:tsz, :], ones_row[0:1, o:o + tsz],
                                             bfc2r_sb[0:1, o2:o2 + w2],
                                             start=False, stop=True)
                        nc.vector.tensor_tensor(out=r1r[:tsz, o2:o2 + w2], in0=ps[:tsz],
                                                in1=r1r[:tsz, o2:o2 + w2], op=ALU.add)
                        nc.sync.dma_start(out=out_d[t0 + o:t0 + o + tsz, o2:o2 + w2],
                                          in_=r1r[:tsz, o2:o2 + w2])
                    return f

                projs = []
                for t, (o, tsz) in enumerate(NT):
                    for j, (o2, w2) in enumerate(SPL_D):
                        projs.append(proj_u(t, o, tsz, o2, w2, j == len(SPL_D) - 1))
                l2 = [ln2_t(t, o, tsz) for t, (o, tsz) in enumerate(NT)]
                fc1s = [fc1_mc(mc) for mc in range(HCH)]
                fc2s = []
                for t, (o, tsz) in enumerate(NT):
                    for j, (o2, w2) in enumerate(SPL_D):
                        fc2s.append(fc2_u(t, o, tsz, o2, w2, j == 0))
                return projs, l2, fc1s, fc2s

            def emit_B(it, q_z, k_sb, vplus, units_a, midblock, units_b):
                ua = list(units_a)
                ub = list(units_b)

                def unit(h):
                    lst = ua if h < 6 else ub
                    if lst:
                        lst.pop(0)()

                attn = work.tile([128, DCH, N], BF16, name=f"attn_{it}", tag="attnbuf", bufs=2)
                pend = [None]

                def flush_norm():
                    if pend[0] is None:
                        return
                    pv, csb, csr, p0, c = pend[0]
                    pend[0] = None
                    bc = psum.tile([128, N], F32, name=f"psbc_{it}_{c}_{csr}", tag="big", bufs=3)
                    for (o, w) in SPL_N:
                        nc.tensor.matmul(bc[:, o:o + w], ones128[csr:csr + 1, :],
                                         csb[csr:csr + 1, o:o + w], start=True, stop=True)
                    rec = work.tile([128, N], F32, name=f"rec_{it}_{c}_{csr}", tag="recbuf", bufs=1)
                    nc.vector.reciprocal_approx_fast(out=rec, in_=bc)
                    nc.vector.tensor_tensor(out=attn[p0:p0 + 64, c, :],
                                            in0=pv[p0:p0 + 64, :] if csr == 64 else pv[64:128, :],
                                            in1=rec[p0:p0 + 64, :],
                                            op=ALU.mult)

                for h in range(H):
                    c = h // 2
                    p0 = 64 * (h % 2)
                    es = []
                    for mt, (mo, msz) in enumerate(NT):
                        ss = psum.tile([128, N], F32, name=f"pss_{it}_{h}_{mt}", tag="big", bufs=3)
                        for (o, w) in SPL_N:
                            nc.tensor.matmul(ss[:, o:o + w],
                                             k_sb[:, c, mo:mo + 128],
                                             q_z[:, h, o:o + w],
                                             start=True, stop=True)
                        if mt == 0:
                            flush_norm()
                        e_sb = work.tile([128, N], BF16, name=f"e_{it}_{h}_{mt}", tag="ebuf", bufs=6)
                        nc.vector.tensor_tensor(out=e_sb, in0=ss,
                                                in1=maskt_sb[:, mt, :], op=ALU.mult)
                        nc.scalar.activation(out=e_sb, in_=e_sb, func=AF.Exp)
                        es.append(e_sb)
                    unit(h)
                    pv = psum.tile([128, N], F32, name=f"pspv_{it}_{h}", tag="big", bufs=3)
                    for mt in range(5):
                        if h % 2 == 0:
                            pv_out = pv[0:65, :]
                            lhs = vplus[mt][0][:, c, 0:65]
                        else:
                            pv_out = pv[0:128, :]
                            lhs = vplus[mt][1][:, c, 0:128]
                        for (o, w) in SPL_N:
                            nc.tensor.matmul(pv_out[:, o:o + w], lhs,
                                             es[mt][:, o:o + w],
                                             start=(mt == 0), stop=(mt == 4))
                    csr = 64 if h % 2 == 0 else 0
                    csb = work.tile([128, N], BF16, name=f"csb_{it}_{h}", tag="csbuf", bufs=2)
                    nc.scalar.activation(out=csb[csr:csr + 1, :], in_=pv[csr:csr + 1, :],
                                         func=AF.Copy)
                    unit(h)
                    pend[0] = (pv, csb, csr, p0, c)
                    if h == 5:
                        flush_norm()
                        for u in ua:
                            u()
                        for u in midblock:
                            u()
                flush_norm()
                for u in ub:
                    u()
                return attn

            projs, l2s, fc1s, fc2s = [], [], [], []
            for it in range(IPC):
                q_sb, k_sb, vplus = emit_A(it)
                if it == 0:
                    nc.sync.dma_start(out=wproj_sb,
                                      in_=wproj_d.rearrange("(c p) m -> p c m", p=128))
                    nc.sync.dma_start(out=wfc2_sb,
                                      in_=wfc2_d.rearrange("(c p) m -> p c m", p=128))
                attn = emit_B(it, q_sb, k_sb, vplus,
                              units_a=projs, midblock=l2s + fc1s, units_b=fc2s)
                projs, l2s, fc1s, fc2s = make_C_units(it, attn)
            for u in projs + l2s + fc1s + fc2s:
                u()

    nc.compile()
    return nc


def prep_in_maps(x, cp_mask, ln1_g, ln1_b, w_qkv, w_proj, b_proj,
                 ln2_g, ln2_b, w_fc1, b_fc1, w_fc2, b_fc2):
    bf = ml_dtypes.bfloat16
    f = np.float32
    x = np.asarray(x, f)
    w_qkv = np.asarray(w_qkv, f)
    w_proj = np.asarray(w_proj, f)
    w_fc1 = np.asarray(w_fc1, f)
    w_fc2 = np.asarray(w_fc2, f)
    g1 = np.asarray(ln1_g, f)
    b1 = np.asarray(ln1_b, f)
    g2 = np.asarray(ln2_g, f)
    b2 = np.asarray(ln2_b, f)

    wqkv_eff = w_qkv * g1[:, None]
    bqkv = b1 @ w_qkv
    scale = DH ** -0.5
    wq = np.ascontiguousarray(wqkv_eff[:, 0:D] * scale).astype(bf)
    wk = np.ascontiguousarray(wqkv_eff[:, D:2 * D]).astype(bf)
    wv = np.ascontiguousarray(wqkv_eff[:, 2 * D:3 * D]).astype(bf)
    bq = (bqkv[0:D] * scale).astype(f)
    bk = bqkv[D:2 * D].astype(f)
    bv = bqkv[2 * D:3 * D]

    bprojr = (np.asarray(b_proj, f) + bv @ w_proj).astype(bf)
    wfc1_eff = (w_fc1 * g2[:, None]).astype(bf)
    bfc1_eff = (np.asarray(b_fc1, f) + b2 @ w_fc1).astype(f)
    bfc2r = np.asarray(b_fc2, f).astype(bf)

    maskt = np.ascontiguousarray(np.asarray(cp_mask, f)[0, 0].T).astype(bf)
    xs = x.reshape(NCORES, TOK, D)

    shared = dict(maskt=maskt, wq=wq, wk=wk, wv=wv, bq=bq, bk=bk,
                  wproj=w_proj.astype(bf), bprojr=bprojr,
                  wfc1=wfc1_eff, bfc1=bfc1_eff,
                  wfc2=w_fc2.astype(bf), bfc2r=bfc2r)
    return [dict(x=np.ascontiguousarray(xs[i]), **shared) for i in range(NCORES)]


_NC_CACHE = {}


def get_nc(use_bias_mm=True):
    key = ("nc", use_bias_mm)
    if key not in _NC_CACHE:
        _NC_CACHE[key] = build_nc(use_bias_mm=use_bias_mm)
    return _NC_CACHE[key]


def run(in_maps, trace=False, **kw):
    need_bias = bool(np.any(in_maps[0]["bprojr"].astype(np.float32))
                     or np.any(in_maps[0]["bfc2r"].astype(np.float32)))
    nc = get_nc(use_bias_mm=need_bias)
    return bass_utils.run_bass_kernel_spmd(nc, in_maps, core_ids=list(range(NCORES)),
                                           trace=trace, **kw)


def kernel(**inputs):
    in_maps = prep_in_maps(**inputs)
    res = run(in_maps)
    out = np.stack([res.results[i]["out"] for i in range(NCORES)])
    return out.reshape(B, N, D).astype(np.float32)

